# revision 11
# baseline (speedup 1.0000x reference)
"""A3C loss kernel for Trainium2 (8 NeuronCores, data-parallel over batch).

The reference is a reverse scan over T=128 timesteps per trajectory:
    R_t   = sum_{s>=t} g^(s-t) r_s + g^(T-t) R0
    gae_t telescopes to adv_t = R_t - v_t   (lambda=1 GAE)
    critic = 0.5 * sum_t adv_t^2
    actor  = -sum_t lp_t * adv_t - beta * sum_{t,a} ent
The suffix scan is a matmul with a [T,T] discount matrix, so the loss is
DMA + one A-reduction + transpose + one matmul per 128-row block.

Trace-driven layout (the stream runs gap-free at ~425 GB/s, so the only
wins left are the edges):
  - values/rewards preloaded as [128, 64, 128] tiles (32KB contiguous
    DRAM per partition), split into 16KB-descriptor halves (measured
    fastest descriptor size).
  - gamma*R0 folded into rewards[:, :, T-1] with one strided gpsimd op.
  - constants (identity, discount matrix) built on-chip during the
    preload so their 512B-line packets stay out of the DMA queue.
  - log_probs/entropies stream in 4-block pairs (16KB/partition per
    dma_start); per-pair gpsimd reduction tree and batched per-quad
    tensor work (4 transposes + 4 matmuls per PSUM bank, one copy/sub)
    keep every engine well under the 9.5us/pair DMA pace and minimize
    instruction bytes (i-fetch shares SDMA engines 64-67 and sets the
    ragged end of the stream).
  - drain: blocks 60-63 load at progressively finer grain (8KB -> 4KB
    -> 2KB descriptors) into dedicated tiles (no pool-recycle deps on
    the DMA queue); block 63 streams in t-halves with the A-reduction
    (gpsimd), entropy sum (scalar) and actor dot (vector) split so the
    compute trailing the last HBM byte is one half-block (~0.9us), not
    a 4-block serialized chain (~4.6us).
  - output staged per 16-block group on the scalar HWDGE ring; group 3
    is stored as blocks 48-61 (overlapped) + a tiny trailing [128,4]
    store so only ~2KB of store receipt sits after the final compute.
"""

import numpy as np
from contextlib import ExitStack

import concourse.bacc as bacc
import concourse.bass as bass
import concourse.tile as tile
from concourse import mybir
from concourse.bass_utils import run_bass_kernel_spmd

GAMMA = 0.99
BETA = 0.01
B, T, A = 65536, 128, 8
N_CORES = 8
BC = B // N_CORES

F32 = mybir.dt.float32
ALU = mybir.AluOpType
ACTF = mybir.ActivationFunctionType


def _blk(t3, k):
    """[128, kb, T] tile -> [128, T] view of block k."""
    try:
        return t3[:, k, :]
    except Exception:
        return t3[:, k : k + 1, :].squeeze(1)


def build_nc(bc: int = BC):
    kb = bc // 128            # 64 blocks of [128, T]
    assert bc % 128 == 0
    NQ = kb // 4              # 16 quads (4 blocks each)
    NPAIR = NQ - 1            # full-rate streamed pairs (blocks 0..59)
    LAGP = 2                  # quads the tensor path runs ahead
    GRP = 16                  # blocks per output store group
    ngrp = kb // GRP

    nc = bacc.Bacc("TRN2", target_bir_lowering=False, debug=False)

    v_d = nc.dram_tensor("values", [bc, T], F32, kind="ExternalInput")
    lv_d = nc.dram_tensor("last_value", [bc], F32, kind="ExternalInput")
    r_d = nc.dram_tensor("rewards", [bc, T], F32, kind="ExternalInput")
    lp_d = nc.dram_tensor("log_probs", [bc, T, A], F32, kind="ExternalInput")
    en_d = nc.dram_tensor("entropies", [bc, T, A], F32, kind="ExternalInput")
    tm_d = nc.dram_tensor("terminal_mask", [bc], mybir.dt.uint8, kind="ExternalInput")
    out_d = nc.dram_tensor("out", [bc, 2], F32, kind="ExternalOutput")

    # partition p owns rows [kb*p, kb*(p+1)): contiguous DRAM per partition
    v3 = v_d.rearrange("(p k) t -> p k t", k=kb)
    r3 = r_d.rearrange("(p k) t -> p k t", k=kb)
    lpq = lp_d.rearrange("(p q g) t a -> q p (g t) a", q=NQ, g=4)
    enq = en_d.rearrange("(p q g) t a -> q p (g t a)", q=NQ, g=4)
    lph = lp_d.rearrange("(p h g) t a -> h p (g t) a", h=2 * NQ, g=2)
    enh = en_d.rearrange("(p h g) t a -> h p (g t a)", h=2 * NQ, g=2)
    lpk = lp_d.rearrange("(p k) t a -> k p t a", k=kb)
    enk = en_d.rearrange("(p k) t a -> k p (t a)", k=kb)
    lv_view = lv_d.rearrange("(p k) -> p k", k=kb)
    tm_view = tm_d.rearrange("(p k) -> p k", k=kb)
    out2 = out_d.rearrange("(p k) j -> p (k j)", k=kb)

    with tile.TileContext(nc) as tc, ExitStack() as ctx:
        singles = ctx.enter_context(tc.tile_pool(name="singles", bufs=1))
        rtp = ctx.enter_context(tc.tile_pool(name="rtp", bufs=1))
        advp = ctx.enter_context(tc.tile_pool(name="advp", bufs=3))
        lp2p = ctx.enter_context(tc.tile_pool(name="lp2p", bufs=2))
        lpp = ctx.enter_context(tc.tile_pool(name="lpp", bufs=2))
        enp = ctx.enter_context(tc.tile_pool(name="enp", bufs=2))
        # s1/s2 are produced and consumed back-to-back on the in-order
        # gpsimd queue, so a single buffer never stalls
        s1p = ctx.enter_context(tc.tile_pool(name="s1p", bufs=1))
        s2p = ctx.enter_context(tc.tile_pool(name="s2p", bufs=1))
        psA = ctx.enter_context(tc.tile_pool(name="psA", bufs=2, space="PSUM"))
        psB = ctx.enter_context(tc.tile_pool(name="psB", bufs=2, space="PSUM"))

        # singles go through SWDGE (gpsimd) so the SP HWDGE FIFO starts on
        # the big loads immediately
        lv_s = singles.tile([128, kb], F32)
        nc.gpsimd.dma_start(out=lv_s, in_=lv_view)
        tm_s = singles.tile([128, kb], mybir.dt.uint8)
        nc.gpsimd.dma_start(out=tm_s, in_=tm_view)

        # SP HWDGE queue order = consumption order: rewards, values, then
        # the lp/en stream.  16KB-descriptor halves stream slightly faster
        # than whole-tensor 32KB descriptors.
        rfull = singles.tile([128, kb, T], F32)
        nc.sync.dma_start(out=rfull[:, : kb // 2, :], in_=r3[:, : kb // 2, :])
        nc.sync.dma_start(out=rfull[:, kb // 2 :, :], in_=r3[:, kb // 2 :, :])
        vfull = singles.tile([128, kb, T], F32)
        nc.sync.dma_start(out=vfull[:, : kb // 2, :], in_=v3[:, : kb // 2, :])
        nc.sync.dma_start(out=vfull[:, kb // 2 :, :], in_=v3[:, kb // 2 :, :])

        # constants built on-chip while the preloads stream:
        # iden[p,x] = (x == p); lgam[s,t] = gamma^(s-t)*(s>=t)
        iden_s = singles.tile([128, 128], F32)
        ones = singles.tile([128, 128], F32)
        nc.vector.memset(ones, 1.0)
        nc.gpsimd.affine_select(
            out=iden_s, in_=ones, pattern=[[1, 128]], base=0,
            channel_multiplier=-1, compare_op=ALU.is_equal, fill=0.0,
        )
        smt = singles.tile([128, 128], mybir.dt.int32)
        nc.gpsimd.iota(smt, pattern=[[-1, 128]], base=0, channel_multiplier=1)
        smtf = singles.tile([128, 128], F32)
        nc.gpsimd.tensor_copy(out=smtf, in_=smt)
        nc.gpsimd.tensor_scalar_mul(smtf, smtf, float(np.log(GAMMA)))
        expf = singles.tile([128, 128], F32)
        nc.scalar.activation(
            out=expf, in_=smtf, func=ACTF.Exp, bias=0.0, scale=1.0
        )
        lgam_s = singles.tile([128, 128], F32)
        nc.gpsimd.affine_select(
            out=lgam_s, in_=expf, pattern=[[-1, 128]], base=0,
            channel_multiplier=1, compare_op=ALU.is_ge, fill=0.0,
        )

        # gr0 = gamma * last_value * (1 - mask)
        tmf = singles.tile([128, kb], F32)
        nc.gpsimd.tensor_copy(out=tmf, in_=tm_s)
        lvm = singles.tile([128, kb], F32)
        nc.gpsimd.tensor_mul(lvm, lv_s, tmf)
        gr0 = singles.tile([128, kb], F32)
        nc.gpsimd.tensor_sub(gr0, lv_s, lvm)
        nc.gpsimd.tensor_scalar_mul(gr0, gr0, GAMMA)

        # fold gamma*R0 into the last timestep of every block at once
        nc.gpsimd.tensor_tensor(
            out=rfull[:, :, T - 1 : T],
            in0=rfull[:, :, T - 1 : T],
            in1=gr0.unsqueeze(2),
            op=ALU.add,
        )

        stage = [
            singles.tile([128, 2 * GRP], F32, name=f"stage{i}") for i in range(ngrp)
        ]
        accs = [singles.tile([128, GRP], F32, name=f"acc{i}") for i in range(ngrp)]
        nbes = [singles.tile([128, GRP], F32, name=f"nbe{i}") for i in range(ngrp)]
        junk = singles.tile([128, 128], F32, name="junk")      # STT main out
        junkE = singles.tile([128, T * A], F32, name="junkE")  # ACT main out

        advq = [None] * NQ

        def early_quad(q):
            # depends only on rewards/values: runs LAGP pairs ahead of the
            # lp/en stream.  4 blocks batched per PSUM bank.
            trp = psA.tile([128, 4, 128], F32)
            for g in range(4):
                nc.tensor.transpose(_blk(trp, g), _blk(rfull, 4 * q + g), iden_s)
            rT = rtp.tile([128, 4, 128], F32)
            nc.vector.tensor_copy(out=rT, in_=trp)
            Rp = psB.tile([128, 4, 128], F32)
            for g in range(4):
                # R[b, t] = sum_s r'T[s, b] * Lgam[s, t]
                nc.tensor.matmul(
                    _blk(Rp, g), lhsT=_blk(rT, g), rhs=lgam_s, start=True, stop=True
                )
            adv = advp.tile([128, 4, 128], F32)
            nc.vector.tensor_sub(adv, Rp, vfull[:, 4 * q : 4 * q + 4, :])
            advq[q] = adv
            for g in range(4):
                k = 4 * q + g
                si, j = k // GRP, k % GRP
                # critic = 0.5 * sum_t adv^2 accumulated straight into the
                # interleaved staging column
                nc.vector.scalar_tensor_tensor(
                    out=junk, in0=_blk(adv, g), scalar=0.5, in1=_blk(adv, g),
                    op0=ALU.mult, op1=ALU.mult,
                    accum_out=stage[si][:, 2 * j + 1 : 2 * j + 2],
                )

        def actor_block(k, adv_g, lp2_slice):
            si, j = k // GRP, k % GRP
            nc.vector.scalar_tensor_tensor(
                out=junk, in0=adv_g, scalar=-1.0, in1=lp2_slice,
                op0=ALU.mult, op1=ALU.mult,
                accum_out=accs[si][:, j : j + 1],
            )

        def ent_block(k, en_slice, w=T * A):
            si, j = k // GRP, k % GRP
            nc.scalar.activation(
                out=junkE[:, :w], in_=en_slice,
                func=ACTF.Copy, bias=0.0, scale=-BETA,
                accum_out=nbes[si][:, j : j + 1],
            )

        def tree(dst, src, n):
            # dst[128, n] = src[128, n, 8].sum(-1) as a pairwise gpsimd tree.
            # s1/s2 are always allocated full-size and sliced so the pools
            # stay single-shape (each distinct shape costs its own slot).
            s1f = s1p.tile([128, 4 * T, 4], F32, name="s1f")
            s1 = s1f[:, :n, :]
            nc.gpsimd.tensor_tensor(
                out=s1, in0=src[:, :, 0:4], in1=src[:, :, 4:8], op=ALU.add
            )
            s2f = s2p.tile([128, 4 * T, 2], F32, name="s2f")
            s2 = s2f[:, :n, :]
            nc.gpsimd.tensor_tensor(
                out=s2, in0=s1[:, :, 0:2], in1=s1[:, :, 2:4], op=ALU.add
            )
            nc.gpsimd.tensor_tensor(
                out=dst.unsqueeze(2), in0=s2[:, :, 0:1], in1=s2[:, :, 1:2],
                op=ALU.add,
            )

        def group_store(si, lo=0, hi=GRP):
            # actor = acc + nbe, interleaved into the staging tile, then the
            # group goes out on the second HWDGE ring (out of the SP FIFO
            # that carries the input loads)
            st3 = stage[si].rearrange("p (j two) -> p j two", two=2)
            nc.vector.tensor_tensor(
                out=st3[:, lo:hi, 0:1],
                in0=accs[si][:, lo:hi].unsqueeze(2),
                in1=nbes[si][:, lo:hi].unsqueeze(2), op=ALU.add,
            )
            nc.scalar.dma_start(
                out=out2[:, si * 2 * GRP + 2 * lo : si * 2 * GRP + 2 * hi],
                in_=stage[si][:, 2 * lo : 2 * hi],
            )

        def stream_pair(p):
            # 4 blocks per dma_start (16KB/partition lines)
            lpb = lpp.tile([128, 4 * T, A], F32, name="lpb")
            nc.sync.dma_start(out=lpb, in_=lpq[p])
            enb = enp.tile([128, 4 * T * A], F32, name="enb")
            nc.sync.dma_start(out=enb, in_=enq[p])

            # lp2[b, (g t)] = sum_a log_probs: pairwise tree on the
            # otherwise-idle gpsimd engine, one tree per pair
            lp2 = lp2p.tile([128, 4 * T], F32)
            tree(lp2, lpb, 4 * T)

            for g in range(4):
                k = 4 * p + g
                ent_block(k, enb[:, g * T * A : (g + 1) * T * A])
                actor_block(k, _blk(advq[p], g), lp2[:, g * T : (g + 1) * T])

            if (p + 1) % 4 == 0:
                group_store((p + 1) // 4 - 1)

        # ---- main loop: blocks 0..59 stream at full descriptor size ----
        for i in range(NQ):
            early_quad(i)
            if LAGP <= i < NPAIR + LAGP:
                stream_pair(i - LAGP)
        for p in range(NQ - LAGP, NPAIR):
            stream_pair(p)

        # ---- drain: blocks 60..63 at progressively finer grain ----
        advT = advq[NQ - 1]
        q0 = 4 * (NQ - 1)          # block 60
        # blocks 60,61 ride one dma_start each (8KB/partition); all drain
        # destinations are dedicated tiles so nothing in the DMA queue
        # waits on pool recycling
        lpT = singles.tile([128, 2 * T, A], F32, name="lpT")
        nc.sync.dma_start(out=lpT, in_=lph[2 * NQ - 2])
        enT = singles.tile([128, 2 * T * A], F32, name="enT")
        nc.sync.dma_start(out=enT, in_=enh[2 * NQ - 2])
        # block 62 alone (4KB/partition)
        lp62t = singles.tile([128, T, A], F32, name="lp62t")
        nc.sync.dma_start(out=lp62t, in_=lpk[q0 + 2])
        en62t = singles.tile([128, T * A], F32, name="en62t")
        nc.sync.dma_start(out=en62t, in_=enk[q0 + 2])
        # block 63 in t-halves (2KB/partition), entropies last
        lp63t = singles.tile([128, T, A], F32, name="lp63t")
        en63t = singles.tile([128, T * A], F32, name="en63t")
        H = T // 2
        nc.sync.dma_start(out=lp63t[:, :H, :], in_=lpk[q0 + 3][:, :H, :])
        nc.sync.dma_start(out=en63t[:, : H * A], in_=enk[q0 + 3][:, : H * A])
        nc.sync.dma_start(out=lp63t[:, H:, :], in_=lpk[q0 + 3][:, H:, :])
        nc.sync.dma_start(out=en63t[:, H * A :], in_=enk[q0 + 3][:, H * A :])

        # blocks 60,61: same shape of work as a steady pair, half size
        lp2T = singles.tile([128, 2 * T], F32, name="lp2T")
        tree(lp2T, lpT, 2 * T)
        for g in range(2):
            k = q0 + g
            ent_block(k, enT[:, g * T * A : (g + 1) * T * A])
            actor_block(k, _blk(advT, g), lp2T[:, g * T : (g + 1) * T])
        # group 3 head (blocks 48..61) stores while block 62/63 stream
        group_store(3, 0, 14)

        # block 62
        lp2_62 = singles.tile([128, T], F32, name="lp2_62")
        tree(lp2_62, lp62t, T)
        ent_block(q0 + 2, en62t)
        actor_block(q0 + 2, _blk(advT, 2), lp2_62)
        st3 = stage[3].rearrange("p (j two) -> p j two", two=2)
        nc.vector.tensor_tensor(
            out=st3[:, 14:15, 0:1], in0=accs[3][:, 14:15].unsqueeze(2),
            in1=nbes[3][:, 14:15].unsqueeze(2), op=ALU.add,
        )

        # block 63: halves; only the second entropy half trails the stream
        lp2_63 = singles.tile([128, T], F32, name="lp2_63")
        tree(lp2_63[:, :H], lp63t[:, :H, :], H)
        nbx = singles.tile([128, 2], F32, name="nbx")
        nc.scalar.activation(
            out=junkE[:, : H * A], in_=en63t[:, : H * A],
            func=ACTF.Copy, bias=0.0, scale=-BETA, accum_out=nbx[:, 0:1],
        )
        tree(lp2_63[:, H:], lp63t[:, H:, :], H)
        actor_block(q0 + 3, _blk(advT, 3), lp2_63)
        # t1 = acc63 + nbe63a on the (now idle) gpsimd engine
        t1 = singles.tile([128, 1], F32, name="t1")
        nc.gpsimd.tensor_tensor(
            out=t1, in0=accs[3][:, 15:16], in1=nbx[:, 0:1], op=ALU.add
        )
        nc.scalar.activation(
            out=junkE[:, H * A : 2 * H * A], in_=en63t[:, H * A :],
            func=ACTF.Copy, bias=0.0, scale=-BETA, accum_out=nbx[:, 1:2],
        )
        nc.vector.tensor_tensor(
            out=st3[:, 15:16, 0:1], in0=t1.unsqueeze(2),
            in1=nbx[:, 1:2].unsqueeze(2), op=ALU.add,
        )
        # tiny trailing store: blocks 62,63 only
        nc.scalar.dma_start(out=out2[:, 6 * GRP + 28 :], in_=stage[3][:, 28:])

    nc.compile()
    return nc


_NC = None


def _get_nc():
    global _NC
    if _NC is None:
        _NC = build_nc(BC)
    return _NC


def _make_in_maps(inputs: dict) -> list[dict]:
    v = np.ascontiguousarray(np.asarray(inputs["values"], dtype=np.float32))
    lv = np.ascontiguousarray(np.asarray(inputs["last_value"], dtype=np.float32))
    r = np.ascontiguousarray(np.asarray(inputs["rewards"], dtype=np.float32))
    lp = np.ascontiguousarray(np.asarray(inputs["log_probs"], dtype=np.float32))
    en = np.ascontiguousarray(np.asarray(inputs["entropies"], dtype=np.float32))
    tm = np.ascontiguousarray(np.asarray(inputs["terminal_mask"]).astype(np.uint8))
    maps = []
    for c in range(N_CORES):
        sl = slice(c * BC, (c + 1) * BC)
        maps.append(
            {
                "values": v[sl],
                "last_value": lv[sl],
                "rewards": r[sl],
                "log_probs": lp[sl],
                "entropies": en[sl],
                "terminal_mask": tm[sl],
            }
        )
    return maps


def _run(inputs: dict, trace: bool = False):
    nc = _get_nc()
    res = run_bass_kernel_spmd(
        nc,
        _make_in_maps(inputs),
        core_ids=list(range(N_CORES)),
        trace=trace,
    )
    out = np.concatenate([res.results[c]["out"] for c in range(N_CORES)], axis=0)
    return out, res


def kernel(**inputs) -> np.ndarray:
    out, _ = _run(inputs, trace=False)
    return out
